# revision 7
# baseline (speedup 1.0000x reference)
"""Llama3 GQA causal attention (B=1, T=2048, D=4096, 32 Q heads / 8 KV heads,
dh=128) on 8 Trainium2 NeuronCores.

Sharding: tensor-parallel over heads. Core i owns KV head i and Q heads
4i..4i+3: Wq/Wk/Wv split column-wise, Wo split row-wise. Each core computes a
partial [T, D] output (rows of Wo for its heads); the host sums the 8 partials.

v2 layout/scheduling notes (vs the original baseline):
 - all matmul operands in fp16 (same PE rate as bf16, ~8x less quantization
   error); PSUM accumulation stays fp32.
 - DMAs are batched: rt streams in [128, 8, 512] pieces (1MB), weights in a
   handful of large transfers. The SP engine issues ~90 DMAs instead of ~360
   (607ns of issue cost each was starving the projection phases).
 - scores are computed in [128, 2, 512] pair tiles so one ACT exp covers 1024
   columns (the 293ns/instr ACT overhead was pacing the attention phases).
 - softmax denominator: probs pairs are accumulated on the DVE (fp16) and
   reduced with 2 matmuls per head instead of one matmul per k-tile (saves
   ~28us of PE time).
 - 1/den via reciprocal_approx_fast (single DVE op) instead of the iterative
   reciprocal (3.3us on one partition).
 - emission interleaves Wo matmul groups into the attention phases as PE
   filler so the PE never waits on the exp chain; phases are software-
   pipelined: P0 P1 A0 P2 A1|W0 P3 A2|W1 A3|W2 W3.
 - PSUM plan: tag "pa" 4x one-bank slots + tag "pb" 2x two-bank slots = 8.
"""

import math
import sys

import numpy as np

sys.path.insert(0, "/opt/trn_rl_repo")

import bass_rust

import concourse.bass as bass
import concourse.mybir as mybir
import concourse.tile as tile
from concourse.bass_utils import run_bass_kernel_spmd

F16 = mybir.dt.float16
F32 = mybir.dt.float32
ACT_COPY = mybir.ActivationFunctionType.Copy
ACT_EXP = mybir.ActivationFunctionType.Exp
ACT_LN = mybir.ActivationFunctionType.Ln

D_MODEL = 4096
N_HEADS = 32
N_KV = 8
DH = 128
T = 2048
NCORES = 8
HQ = N_HEADS // NCORES  # 4 q heads per core
NT = T // 128  # 16 k tiles
NCH = T // 512  # 4 q chunks
SCALE = 1.0 / math.sqrt(DH)
ROPE = dict(
    rope_theta=500000.0,
    factor=32.0,
    hi_freq_factor=4.0,
    lo_freq_factor=1.0,
    original_context_length=8192,
)


def _rope_tables():
    """cos/sin tables in transposed layout [dh, T]; sin has the rotate-half
    sign folded in (rows 0:64 negated)."""
    idx = np.arange(0, DH, 2, dtype=np.float64) / DH
    freq = (1.0 / (2.0 * math.pi)) * ROPE["rope_theta"] ** (-idx)
    factor, lo, hi = ROPE["factor"], ROPE["lo_freq_factor"], ROPE["hi_freq_factor"]
    L0 = ROPE["original_context_length"]
    freq_low, freq_high = lo / L0, hi / L0
    freq_scaled = np.where(freq < freq_low, freq / factor, freq)
    smooth = np.clip((L0 * freq - lo) / (hi - lo), 0.0, 1.0)
    freq_smooth = (1.0 - smooth) * (freq / factor) + smooth * freq
    is_mid = (freq >= freq_low) & (freq <= freq_high)
    freq = np.where(is_mid, freq_smooth, freq_scaled)
    pos = np.arange(T, dtype=np.float64)
    phase = 2.0 * math.pi * pos[:, None] * freq[None, :]  # [T, 64]
    emb = np.concatenate([phase, phase], axis=-1)  # [T, 128]
    cos = np.cos(emb).astype(np.float32)
    sin = np.sin(emb).astype(np.float32)
    cosT = np.ascontiguousarray(cos.T)  # [128, T]
    sinT = np.ascontiguousarray(sin.T)
    sinM = sinT.copy()
    sinM[:64] = -sinT[:64]
    return cosT, sinM


def _build_nc():
    nc = bass.Bass()
    rt = nc.dram_tensor("rt", [D_MODEL, T], F16, kind="ExternalInput")
    wq = nc.dram_tensor("wq", [D_MODEL, HQ * DH], F16, kind="ExternalInput")
    wk = nc.dram_tensor("wk", [D_MODEL, DH], F16, kind="ExternalInput")
    wv = nc.dram_tensor("wv", [D_MODEL, DH], F16, kind="ExternalInput")
    wo = nc.dram_tensor("wo", [HQ * DH, D_MODEL], F16, kind="ExternalInput")
    # duplicated-pair rope tables: [:, c, hf, :] = table[:, 512c:512c+512]
    cos2 = nc.dram_tensor("cos2", [DH, NCH, 2, 512], F16, kind="ExternalInput")
    sin2 = nc.dram_tensor("sin2", [DH, NCH, 2, 512], F16, kind="ExternalInput")
    # pair masks msk[:, rp, hf, :] for diagonal pair rp, half hf
    msk = nc.dram_tensor("msk", [128, 2, 2, 512], F16, kind="ExternalInput")
    iden = nc.dram_tensor("iden", [128, 128], F16, kind="ExternalInput")
    onesk = nc.dram_tensor("onesk", [128, 1], F16, kind="ExternalInput")
    onesf = nc.dram_tensor("onesf", [1, 128], F32, kind="ExternalInput")
    outp = nc.dram_tensor("outp", [T, D_MODEL], F16, kind="ExternalOutput")

    rt3 = rt.rearrange("(o p) t -> p o t", p=128)  # [128, 32, T]
    wq3 = wq.rearrange("(o p) m -> p o m", p=128)  # [128, 32, 512]
    wk3 = wk.rearrange("(o p) m -> p o m", p=128)  # [128, 32, 128]
    wv3 = wv.rearrange("(o p) m -> p o m", p=128)
    wo3 = wo.rearrange("(o p) n -> p o n", p=128)  # [128, 4, 4096]

    with tile.TileContext(nc) as tc:
        with (
            tc.tile_pool(name="consts", bufs=1) as cpool,
            tc.tile_pool(name="acts", bufs=1) as apool,
            tc.tile_pool(name="rtp", bufs=3) as rpool,
            tc.tile_pool(name="wrk", bufs=4) as wpool,
            tc.tile_pool(name="p2p", bufs=3) as p2pool,
            tc.tile_pool(name="pap", bufs=2) as papool,
            tc.tile_pool(name="sml", bufs=2) as spool,
            tc.tile_pool(name="stgp", bufs=2) as opool,
            tc.tile_pool(name="ps", bufs=4, space="PSUM") as ps,
        ):
            # ---- resident constants / weights (few big DMAs) ----
            iden_sb = cpool.tile([128, 128], F16)
            nc.sync.dma_start(iden_sb, iden[:, :])
            onesk_sb = cpool.tile([128, 1], F16)
            nc.sync.dma_start(onesk_sb, onesk[:, :])
            onesf_sb = cpool.tile([1, 128], F32)
            nc.sync.dma_start(onesf_sb, onesf[:, :])
            msk_sb = cpool.tile([128, 2, 2, 512], F16)
            nc.sync.dma_start(msk_sb, msk[:, :, :, :])

            wq_sb = cpool.tile([128, 32, HQ * DH], F16)
            wk_sb = cpool.tile([128, 32, DH], F16)
            wv_sb = cpool.tile([128, 32, DH], F16)
            wo_sb = cpool.tile([128, HQ, D_MODEL], F16)
            cos_sb = cpool.tile([DH, NCH, 2, 512], F16)
            sin_sb = cpool.tile([DH, NCH, 2, 512], F16)

            # activations that persist across phases
            qt_sb = apool.tile([128, HQ, T], F16)  # Q^T per head, rope'd
            kt_sb = apool.tile([128, T], F16)  # K^T, rope'd
            v_sb = apool.tile([128, NT, DH], F16)  # V tiles [tk, j, dh]
            cx_sb = apool.tile([128, HQ, T], F16)  # normalized ctx^T

            # ---- rt piece DMA bookkeeping ----
            pieces = {}

            def rt_dma(p):
                if p >= 16:
                    return
                c, g = p // 4, p % 4
                t = rpool.tile([128, 8, 512], F16, tag="rtp", name=f"rt{c}_{g}")
                nc.sync.dma_start(t, rt3[:, 8 * g : 8 * (g + 1), 512 * c : 512 * (c + 1)])
                pieces[p] = t

            # preamble: weights go on the scalar-engine HWDGE ring so they
            # stream in parallel with the rt pieces on the sync ring (HWDGE
            # DMAs are FIFO per issuing engine)
            for q in range(4):
                nc.scalar.dma_start(
                    wq_sb[:, 8 * q : 8 * (q + 1), :],
                    wq3[:, 8 * q : 8 * (q + 1), :],
                )
                if q == 0:
                    nc.scalar.dma_start(wk_sb[:, 0:8, :], wk3[:, 0:8, :])
                    nc.scalar.dma_start(wv_sb[:, 0:8, :], wv3[:, 0:8, :])
            nc.scalar.dma_start(wk_sb[:, 8:32, :], wk3[:, 8:32, :])
            nc.scalar.dma_start(wv_sb[:, 8:32, :], wv3[:, 8:32, :])
            nc.scalar.dma_start(cos_sb, cos2[:, :, :, :])
            nc.scalar.dma_start(sin_sb, sin2[:, :, :, :])
            rt_dma(0)
            rt_dma(1)
            rt_dma(2)

            # PE warm-up: ~28 junk matmuls on tiny early-arriving constants
            # fill the initial DMA wait and flip the HAM clock gate to 8/8
            # before the first real matmul lands.
            junk_ps = ps.tile([128, 2, 512], F32, tag="pb", bufs=2, name="junk")
            mskf = msk_sb.rearrange("p a b f -> p (a b f)")
            for w in range(28):
                nc.tensor.matmul(
                    junk_ps[:, w % 2, :],
                    iden_sb,
                    mskf[:, 512 * (w % 4) : 512 * (w % 4 + 1)],
                    start=True,
                    stop=True,
                )

            # ---- wo filler queue ----
            class WoQueue:
                def __init__(self, c):
                    self.c = c
                    self.items = [(4 * c + s, n) for s in range(4) for n in range(8)]
                    self.i = 0
                    self.stg = None

                def done(self):
                    return self.i >= len(self.items)

                def emit(self, k):
                    for _ in range(k):
                        if self.done():
                            return
                        tq, n = self.items[self.i]
                        self.i += 1
                        if n % 4 == 0:
                            self.stg = opool.tile(
                                [128, 2048], F16, tag="stg", name=f"stg{tq}_{n}"
                            )
                        wps = ps.tile(
                            [128, 512], F32, tag="pa", bufs=4, name=f"wo{tq}_{n}"
                        )
                        for h in range(HQ):
                            nc.tensor.matmul(
                                wps,
                                cx_sb[:, h, 128 * tq : 128 * (tq + 1)],
                                wo_sb[:, h, 512 * n : 512 * (n + 1)],
                                start=(h == 0),
                                stop=(h == HQ - 1),
                            )
                        dst = self.stg[:, 512 * (n % 4) : 512 * (n % 4 + 1)]
                        if n % 2 == 0:
                            nc.scalar.activation(dst, wps, ACT_COPY)
                        else:
                            nc.vector.tensor_copy(dst, wps)
                        if n % 4 == 3:
                            nc.sync.dma_start(
                                outp[
                                    128 * tq : 128 * (tq + 1),
                                    2048 * (n // 4) : 2048 * (n // 4 + 1),
                                ],
                                self.stg,
                            )

                def flush(self):
                    self.emit(len(self.items) - self.i)

            # ---- projection chunk ----
            def proj_chunk(c):
                bq01 = ps.tile([128, 2, 512], F32, tag="pb", bufs=2, name=f"bq01_{c}")
                bq23 = ps.tile([128, 2, 512], F32, tag="pb", bufs=2, name=f"bq23_{c}")
                kacc = ps.tile([128, 512], F32, tag="pa", bufs=4, name=f"kacc{c}")
                vacc = ps.tile([128, 512], F32, tag="pa", bufs=4, name=f"vacc{c}")
                accs = [bq01[:, 0, :], bq01[:, 1, :], bq23[:, 0, :], bq23[:, 1, :]]
                for g in range(4):
                    pc = pieces[4 * c + g]
                    for oo in range(8):
                        o = 8 * g + oo
                        rtt = pc[:, oo, :]
                        st, sp = (o == 0), (o == 31)
                        for h in range(HQ):
                            nc.tensor.matmul(
                                accs[h],
                                wq_sb[:, o, 128 * h : 128 * (h + 1)],
                                rtt,
                                start=st,
                                stop=sp,
                            )
                        nc.tensor.matmul(kacc, wk_sb[:, o, :], rtt, start=st, stop=sp)
                        nc.tensor.matmul(vacc, wv_sb[:, o, :], rtt, start=st, stop=sp)
                    rt_dma(4 * c + g + 3)
                # RoPE on the two q pair-accs (fp16 work tiles)
                for bi, big in enumerate([bq01, bq23]):
                    xh = wpool.tile([128, 2, 512], F16, tag="wrk", name=f"xh{c}_{bi}")
                    nc.scalar.activation(xh, big, ACT_COPY)
                    xs = wpool.tile([128, 2, 512], F16, tag="wrk", name=f"xs{c}_{bi}")
                    nc.vector.tensor_copy(xs[0:64, :, :], xh[64:128, :, :])
                    nc.vector.tensor_copy(xs[64:128, :, :], xh[0:64, :, :])
                    nc.vector.tensor_mul(xh, xh, cos_sb[:, c, :, :])
                    nc.vector.tensor_mul(xs, xs, sin_sb[:, c, :, :])
                    nc.vector.tensor_add(
                        qt_sb[:, 2 * bi : 2 * bi + 2, 512 * c : 512 * (c + 1)], xh, xs
                    )
                # RoPE on k (half-size work tiles share the wrk tag)
                xkh = wpool.tile([128, 512], F16, tag="wrk", name=f"xkh{c}")
                nc.scalar.activation(xkh, kacc, ACT_COPY)
                xks = wpool.tile([128, 512], F16, tag="wrk", name=f"xks{c}")
                nc.vector.tensor_copy(xks[0:64, :], xkh[64:128, :])
                nc.vector.tensor_copy(xks[64:128, :], xkh[0:64, :])
                nc.vector.tensor_mul(xkh, xkh, cos_sb[:, c, 0, :])
                nc.vector.tensor_mul(xks, xks, sin_sb[:, c, 0, :])
                nc.vector.tensor_add(kt_sb[:, 512 * c : 512 * (c + 1)], xkh, xks)
                # V^T -> V via PE transpose (4 x 128x128)
                vt = wpool.tile([128, 512], F16, tag="wrk", name=f"vt{c}")
                nc.scalar.activation(vt, vacc, ACT_COPY)
                for s in range(4):
                    ptr = ps.tile([128, 128], F16, tag="pa", bufs=4, name=f"vtr{c}_{s}")
                    nc.tensor.transpose(ptr, vt[:, 128 * s : 128 * (s + 1)], iden_sb)
                    nc.vector.tensor_copy(v_sb[:, 4 * c + s, :], ptr)

            # ---- attention chunk (with optional wo filler) ----
            def attn_chunk(c, filler):
                cs = slice(512 * c, 512 * (c + 1))
                npair = 2 * (c + 1)
                delayed = []  # deferred den/normalize thunks
                if filler is not None:
                    # bridge the rope-cast stall at the proj->attn boundary
                    # with independent PE work (keeps the HAM clock warm)
                    filler.emit(2)

                def run_delayed():
                    if delayed:
                        delayed.pop(0)()

                def mk_den(h, pacc, ctx_ps):
                    def den_thunk():
                        den_ps = ps.tile(
                            [1, 512], F32, tag="pa", bufs=4, name=f"den{c}_{h}"
                        )
                        nc.tensor.matmul(
                            den_ps, onesk_sb, pacc[:, 0, :], start=True, stop=False
                        )
                        nc.tensor.matmul(
                            den_ps, onesk_sb, pacc[:, 1, :], start=False, stop=True
                        )
                        # 1/den = exp(-ln(den)): Ln and Exp share one ACT
                        # table set, and the rank-1 matmul broadcasts ln(den)
                        # across partitions before the (vectorized) Exp.
                        lnden = spool.tile(
                            [1, 512], F32, tag="den", name=f"lnden{c}_{h}"
                        )
                        nc.scalar.activation(lnden, den_ps, ACT_LN)

                        def fin_thunk():
                            bc_ps = ps.tile(
                                [128, 512], F32, tag="pa", bufs=4, name=f"bc{c}_{h}"
                            )
                            nc.tensor.matmul(
                                bc_ps, onesf_sb, lnden, start=True, stop=True
                            )
                            bc_sb = spool.tile(
                                [128, 512], F32, tag="bcb", name=f"bcsb{c}_{h}"
                            )
                            nc.scalar.activation(bc_sb, bc_ps, ACT_EXP, scale=-1.0)
                            nc.vector.tensor_mul(cx_sb[:, h, cs], ctx_ps, bc_sb)

                        delayed.append(fin_thunk)

                    delayed.append(den_thunk)

                for h in range(HQ):
                    ctx_ps = ps.tile(
                        [128, 512], F32, tag="pa", bufs=4, name=f"ctx{c}_{h}"
                    )
                    pacc = papool.tile(
                        [128, 2, 512], F16, tag="pacc", name=f"pacc{c}_{h}"
                    )
                    prev = None
                    for pi in range(npair):
                        s2 = ps.tile(
                            [128, 2, 512], F32, tag="pb", bufs=2, name=f"s{c}_{h}_{pi}"
                        )
                        for hf in range(2):
                            j = 2 * pi + hf
                            nc.tensor.matmul(
                                s2[:, hf, :],
                                kt_sb[:, 128 * j : 128 * (j + 1)],
                                qt_sb[:, h, cs],
                                start=True,
                                stop=True,
                            )
                        p2 = p2pool.tile([128, 2, 512], F16, tag="p2", name=f"p{c}_{h}_{pi}")
                        nc.scalar.activation(p2, s2, ACT_EXP, scale=SCALE)
                        rp = pi - 2 * c
                        if rp >= 0:  # diagonal pair: causal mask
                            nc.vector.tensor_mul(p2, p2, msk_sb[:, rp, :, :])
                        if pi == 0:
                            nc.vector.tensor_copy(pacc, p2)
                        else:
                            nc.vector.tensor_add(pacc, pacc, p2)
                        if prev is not None:
                            pp2, ppi = prev
                            for hf in range(2):
                                j = 2 * ppi + hf
                                nc.tensor.matmul(
                                    ctx_ps,
                                    v_sb[:, j, :],
                                    pp2[:, hf, :],
                                    start=(j == 0),
                                    stop=False,
                                )
                        prev = (p2, pi)
                        if filler is not None:
                            filler.emit(1)
                        run_delayed()
                    pp2, ppi = prev
                    for hf in range(2):
                        j = 2 * ppi + hf
                        nc.tensor.matmul(
                            ctx_ps,
                            v_sb[:, j, :],
                            pp2[:, hf, :],
                            start=(j == 0),
                            stop=(hf == 1),
                        )
                    mk_den(h, pacc, ctx_ps)
                    if filler is not None:
                        filler.emit(1)
                    run_delayed()
                # drain deferred chains, with filler between to keep PE fed
                while delayed:
                    if filler is not None:
                        filler.emit(1)
                    run_delayed()
                if filler is not None:
                    filler.flush()

            # ---- emission: software-pipelined phases ----
            proj_chunk(0)
            # wo weights: overlap their DMA with chunk-1 compute
            nc.sync.dma_start(wo_sb[:, 0:2, :], wo3[:, 0:2, :])
            proj_chunk(1)
            nc.sync.dma_start(wo_sb[:, 2:4, :], wo3[:, 2:4, :])
            attn_chunk(0, None)
            proj_chunk(2)
            attn_chunk(1, WoQueue(0))
            proj_chunk(3)
            attn_chunk(2, WoQueue(1))
            attn_chunk(3, WoQueue(2))
            w3 = WoQueue(3)
            w3.flush()

    # TRN2 allows at most 1 sem wait per instruction; split the extras into
    # EventSemaphore chains (same pass bacc.compile runs).
    bass_rust.generate_event_semaphores(nc)
    return nc


_NC = None


def _get_nc():
    global _NC
    if _NC is None:
        _NC = _build_nc()
    return _NC


def _host_inputs(resid, Wq, Wk, Wv, Wo):
    f16 = np.float16
    r2 = np.asarray(resid, dtype=np.float32).reshape(T, D_MODEL)
    rt = np.ascontiguousarray(r2.T).astype(f16)  # [D, T]
    cosT, sinM = _rope_tables()
    cos2 = np.empty((DH, NCH, 2, 512), np.float32)
    sin2 = np.empty((DH, NCH, 2, 512), np.float32)
    for c in range(NCH):
        for hf in range(2):
            cos2[:, c, hf, :] = cosT[:, 512 * c : 512 * (c + 1)]
            sin2[:, c, hf, :] = sinM[:, 512 * c : 512 * (c + 1)]
    cos2 = cos2.astype(f16)
    sin2 = sin2.astype(f16)
    # pair masks: msk[part, rp, hf, u] = part <= u - 128*(2*rp+hf)
    u = np.arange(512)[None, :]
    p = np.arange(128)[:, None]
    msk = np.empty((128, 2, 2, 512), np.float32)
    for rp in range(2):
        for hf in range(2):
            msk[:, rp, hf, :] = (p <= u - 128 * (2 * rp + hf)).astype(np.float32)
    msk = msk.astype(f16)
    iden = np.eye(128, dtype=f16)
    onesk = np.ones((128, 1), f16)
    onesf = np.ones((1, 128), np.float32)
    Wq = np.asarray(Wq, np.float32)
    Wk = np.asarray(Wk, np.float32)
    Wv = np.asarray(Wv, np.float32)
    Wo = np.asarray(Wo, np.float32)
    in_maps = []
    for i in range(NCORES):
        in_maps.append(
            {
                "rt": rt,
                "wq": np.ascontiguousarray(Wq[:, 512 * i : 512 * (i + 1)]).astype(f16),
                "wk": np.ascontiguousarray(Wk[:, 128 * i : 128 * (i + 1)]).astype(f16),
                "wv": np.ascontiguousarray(Wv[:, 128 * i : 128 * (i + 1)]).astype(f16),
                "wo": np.ascontiguousarray(Wo[512 * i : 512 * (i + 1), :]).astype(f16),
                "cos2": cos2,
                "sin2": sin2,
                "msk": msk,
                "iden": iden,
                "onesk": onesk,
                "onesf": onesf,
            }
        )
    return in_maps


def run(resid, Wq, Wk, Wv, Wo, **spmd_kwargs):
    in_maps = _host_inputs(resid, Wq, Wk, Wv, Wo)
    nc = _get_nc()
    res = run_bass_kernel_spmd(nc, in_maps, core_ids=list(range(NCORES)), **spmd_kwargs)
    out = np.zeros((T, D_MODEL), np.float32)
    for rmap in res.results:
        out += rmap["outp"].astype(np.float32)
    return out.reshape(1, T, D_MODEL), res


def kernel(resid, Wq, Wk, Wv, Wo):
    out, _ = run(resid, Wq, Wk, Wv, Wo)
    return out


# revision 12
# speedup vs baseline: 1.0416x; 1.0416x over previous
"""Llama3 GQA causal attention (B=1, T=2048, D=4096, 32 Q heads / 8 KV heads,
dh=128) on 8 Trainium2 NeuronCores.

Sharding: tensor-parallel over heads. Core i owns KV head i and Q heads
4i..4i+3: Wq/Wk/Wv split column-wise, Wo split row-wise. Each core computes a
partial [T, D] output (rows of Wo for its heads); the host sums the 8 partials.

v2 layout/scheduling notes (vs the original baseline):
 - all matmul operands in fp16 (same PE rate as bf16, ~8x less quantization
   error); PSUM accumulation stays fp32.
 - DMAs are batched: rt streams in [128, 8, 512] pieces (1MB), weights in a
   handful of large transfers. The SP engine issues ~90 DMAs instead of ~360
   (607ns of issue cost each was starving the projection phases).
 - scores are computed in [128, 2, 512] pair tiles so one ACT exp covers 1024
   columns (the 293ns/instr ACT overhead was pacing the attention phases).
 - softmax denominator: probs pairs are accumulated on the DVE (fp16) and
   reduced with 2 matmuls per head instead of one matmul per k-tile (saves
   ~28us of PE time).
 - 1/den via reciprocal_approx_fast (single DVE op) instead of the iterative
   reciprocal (3.3us on one partition).
 - emission interleaves Wo matmul groups into the attention phases as PE
   filler so the PE never waits on the exp chain; phases are software-
   pipelined: P0 P1 A0 P2 A1|W0 P3 A2|W1 A3|W2 W3.
 - PSUM plan: tag "pa" 4x one-bank slots + tag "pb" 2x two-bank slots = 8.
"""

import math
import sys

import numpy as np

sys.path.insert(0, "/opt/trn_rl_repo")

import bass_rust

import concourse.bass as bass
import concourse.mybir as mybir
import concourse.tile as tile
from concourse.bass_utils import run_bass_kernel_spmd

F16 = mybir.dt.float16
F32 = mybir.dt.float32
ACT_COPY = mybir.ActivationFunctionType.Copy
ACT_EXP = mybir.ActivationFunctionType.Exp
ACT_LN = mybir.ActivationFunctionType.Ln

D_MODEL = 4096
N_HEADS = 32
N_KV = 8
DH = 128
T = 2048
NCORES = 8
HQ = N_HEADS // NCORES  # 4 q heads per core
NT = T // 128  # 16 k tiles
NCH = T // 512  # 4 q chunks
SCALE = 1.0 / math.sqrt(DH)
ROPE = dict(
    rope_theta=500000.0,
    factor=32.0,
    hi_freq_factor=4.0,
    lo_freq_factor=1.0,
    original_context_length=8192,
)


def _rope_tables():
    """cos/sin tables in transposed layout [dh, T]; sin has the rotate-half
    sign folded in (rows 0:64 negated)."""
    idx = np.arange(0, DH, 2, dtype=np.float64) / DH
    freq = (1.0 / (2.0 * math.pi)) * ROPE["rope_theta"] ** (-idx)
    factor, lo, hi = ROPE["factor"], ROPE["lo_freq_factor"], ROPE["hi_freq_factor"]
    L0 = ROPE["original_context_length"]
    freq_low, freq_high = lo / L0, hi / L0
    freq_scaled = np.where(freq < freq_low, freq / factor, freq)
    smooth = np.clip((L0 * freq - lo) / (hi - lo), 0.0, 1.0)
    freq_smooth = (1.0 - smooth) * (freq / factor) + smooth * freq
    is_mid = (freq >= freq_low) & (freq <= freq_high)
    freq = np.where(is_mid, freq_smooth, freq_scaled)
    pos = np.arange(T, dtype=np.float64)
    phase = 2.0 * math.pi * pos[:, None] * freq[None, :]  # [T, 64]
    emb = np.concatenate([phase, phase], axis=-1)  # [T, 128]
    cos = np.cos(emb).astype(np.float32)
    sin = np.sin(emb).astype(np.float32)
    cosT = np.ascontiguousarray(cos.T)  # [128, T]
    sinT = np.ascontiguousarray(sin.T)
    sinM = sinT.copy()
    sinM[:64] = -sinT[:64]
    return cosT, sinM


def _build_nc():
    nc = bass.Bass()
    rt = nc.dram_tensor("rt", [D_MODEL, T], F16, kind="ExternalInput")
    wq = nc.dram_tensor("wq", [D_MODEL, HQ * DH], F16, kind="ExternalInput")
    wk = nc.dram_tensor("wk", [D_MODEL, DH], F16, kind="ExternalInput")
    wv = nc.dram_tensor("wv", [D_MODEL, DH], F16, kind="ExternalInput")
    wo = nc.dram_tensor("wo", [HQ * DH, D_MODEL], F16, kind="ExternalInput")
    # duplicated-pair rope tables: [:, c, hf, :] = table[:, 512c:512c+512]
    cos2 = nc.dram_tensor("cos2", [DH, NCH, 2, 512], F16, kind="ExternalInput")
    sin2 = nc.dram_tensor("sin2", [DH, NCH, 2, 512], F16, kind="ExternalInput")
    # pair masks msk[:, rp, hf, :] for diagonal pair rp, half hf
    msk = nc.dram_tensor("msk", [128, 2, 2, 512], F16, kind="ExternalInput")
    iden = nc.dram_tensor("iden", [128, 128], F16, kind="ExternalInput")
    onesk = nc.dram_tensor("onesk", [128, 1], F16, kind="ExternalInput")
    onesf = nc.dram_tensor("onesf", [1, 128], F32, kind="ExternalInput")
    outp = nc.dram_tensor("outp", [T, D_MODEL], F16, kind="ExternalOutput")

    rt3 = rt.rearrange("(o p) t -> p o t", p=128)  # [128, 32, T]
    wq3 = wq.rearrange("(o p) m -> p o m", p=128)  # [128, 32, 512]
    wk3 = wk.rearrange("(o p) m -> p o m", p=128)  # [128, 32, 128]
    wv3 = wv.rearrange("(o p) m -> p o m", p=128)
    wo3 = wo.rearrange("(o p) n -> p o n", p=128)  # [128, 4, 4096]

    with tile.TileContext(nc) as tc:
        with (
            tc.tile_pool(name="consts", bufs=1) as cpool,
            tc.tile_pool(name="acts", bufs=1) as apool,
            tc.tile_pool(name="rtp", bufs=3) as rpool,
            tc.tile_pool(name="wrk", bufs=4) as wpool,
            tc.tile_pool(name="p2p", bufs=3) as p2pool,
            tc.tile_pool(name="pap", bufs=2) as papool,
            tc.tile_pool(name="sml", bufs=2) as spool,
            tc.tile_pool(name="stgp", bufs=2) as opool,
            tc.tile_pool(name="ps", bufs=4, space="PSUM") as ps,
        ):
            # ---- resident constants / weights (few big DMAs) ----
            iden_sb = cpool.tile([128, 128], F16)
            nc.sync.dma_start(iden_sb, iden[:, :])
            onesk_sb = cpool.tile([128, 1], F16)
            nc.sync.dma_start(onesk_sb, onesk[:, :])
            onesf_sb = cpool.tile([1, 128], F32)
            nc.sync.dma_start(onesf_sb, onesf[:, :])
            msk_sb = cpool.tile([128, 2, 2, 512], F16)
            nc.sync.dma_start(msk_sb, msk[:, :, :, :])

            wq_sb = cpool.tile([128, 32, HQ * DH], F16)
            wk_sb = cpool.tile([128, 32, DH], F16)
            wv_sb = cpool.tile([128, 32, DH], F16)
            wo_sb = cpool.tile([128, HQ, D_MODEL], F16)
            cos_sb = cpool.tile([DH, NCH, 2, 512], F16)
            sin_sb = cpool.tile([DH, NCH, 2, 512], F16)

            # activations that persist across phases
            qt_sb = apool.tile([128, HQ, T], F16)  # Q^T per head, rope'd
            kt_sb = apool.tile([128, T], F16)  # K^T, rope'd
            v_sb = apool.tile([128, NT, DH], F16)  # V tiles [tk, j, dh]
            cx_sb = apool.tile([128, HQ, T], F16)  # normalized ctx^T

            # ---- rt piece DMA bookkeeping ----
            pieces = {}

            def rt_dma(p):
                if p >= 16:
                    return
                c, g = p // 4, p % 4
                t = rpool.tile([128, 8, 512], F16, tag="rtp", name=f"rt{c}_{g}")
                nc.sync.dma_start(t, rt3[:, 8 * g : 8 * (g + 1), 512 * c : 512 * (c + 1)])
                pieces[p] = t

            # preamble: the sync ring streams exactly what the o-loop needs in
            # o-group-major order (wq/wk/wv slices + the rt piece per 8-o
            # group); rope tables and wo ride the scalar ring so they don't
            # delay the critical stream.
            for g in range(4):
                nc.sync.dma_start(
                    wq_sb[:, 8 * g : 8 * (g + 1), :], wq3[:, 8 * g : 8 * (g + 1), :]
                )
                nc.sync.dma_start(
                    wk_sb[:, 8 * g : 8 * (g + 1), :], wk3[:, 8 * g : 8 * (g + 1), :]
                )
                nc.sync.dma_start(
                    wv_sb[:, 8 * g : 8 * (g + 1), :], wv3[:, 8 * g : 8 * (g + 1), :]
                )
                if g < 3:
                    rt_dma(g)
            nc.scalar.dma_start(cos_sb, cos2[:, :, :, :])
            nc.scalar.dma_start(sin_sb, sin2[:, :, :, :])

            # PE warm-up: junk matmuls on tiny early-arriving constants fill
            # the initial DMA wait and flip the HAM clock gate to 8/8 before
            # the first real matmul lands.
            junk_ps = ps.tile([128, 2, 512], F32, tag="pb", bufs=2, name="junk")
            mskf = msk_sb.rearrange("p a b f -> p (a b f)")
            for w in range(16):
                nc.tensor.matmul(
                    junk_ps[:, w % 2, :],
                    iden_sb,
                    mskf[:, 512 * (w % 4) : 512 * (w % 4 + 1)],
                    start=True,
                    stop=True,
                )

            # ---- wo filler queue ----
            class WoQueue:
                def __init__(self, c):
                    self.c = c
                    self.items = [(4 * c + s, n) for s in range(4) for n in range(8)]
                    self.i = 0
                    self.stg = None

                def done(self):
                    return self.i >= len(self.items)

                def emit(self, k):
                    for _ in range(k):
                        if self.done():
                            return
                        tq, n = self.items[self.i]
                        self.i += 1
                        if n % 4 == 0:
                            self.stg = opool.tile(
                                [128, 2048], F16, tag="stg", name=f"stg{tq}_{n}"
                            )
                        wps = ps.tile(
                            [128, 512], F32, tag="pa", bufs=4, name=f"wo{tq}_{n}"
                        )
                        for h in range(HQ):
                            nc.tensor.matmul(
                                wps,
                                cx_sb[:, h, 128 * tq : 128 * (tq + 1)],
                                wo_sb[:, h, 512 * n : 512 * (n + 1)],
                                start=(h == 0),
                                stop=(h == HQ - 1),
                            )
                        dst = self.stg[:, 512 * (n % 4) : 512 * (n % 4 + 1)]
                        if n % 2 == 0:
                            nc.scalar.activation(dst, wps, ACT_COPY)
                        else:
                            nc.vector.tensor_copy(dst, wps)
                        if n % 4 == 3:
                            nc.sync.dma_start(
                                outp[
                                    128 * tq : 128 * (tq + 1),
                                    2048 * (n // 4) : 2048 * (n // 4 + 1),
                                ],
                                self.stg,
                            )

                def flush(self):
                    self.emit(len(self.items) - self.i)

            # ---- projection chunk ----
            def proj_chunk(c, micro=None):
                bq01 = ps.tile([128, 2, 512], F32, tag="pb", bufs=2, name=f"bq01_{c}")
                bq23 = ps.tile([128, 2, 512], F32, tag="pb", bufs=2, name=f"bq23_{c}")
                kacc = ps.tile([128, 512], F32, tag="pa", bufs=4, name=f"kacc{c}")
                vacc = ps.tile([128, 512], F32, tag="pa", bufs=4, name=f"vacc{c}")
                accs = [bq01[:, 0, :], bq01[:, 1, :], bq23[:, 0, :], bq23[:, 1, :]]

                def rope_q(bi):
                    big = [bq01, bq23][bi]
                    xh = wpool.tile(
                        [128, 2, 512], F16, tag="wrk", name=f"xh{c}_{bi}"
                    )
                    nc.scalar.activation(xh, big, ACT_COPY)
                    xs = wpool.tile(
                        [128, 2, 512], F16, tag="wrk", name=f"xs{c}_{bi}"
                    )
                    nc.vector.tensor_copy(xs[0:64, :, :], xh[64:128, :, :])
                    nc.vector.tensor_copy(xs[64:128, :, :], xh[0:64, :, :])
                    nc.vector.tensor_mul(xh, xh, cos_sb[:, c, :, :])
                    nc.vector.tensor_mul(xs, xs, sin_sb[:, c, :, :])
                    nc.vector.tensor_add(
                        qt_sb[:, 2 * bi : 2 * bi + 2, 512 * c : 512 * (c + 1)], xh, xs
                    )

                def rope_k():
                    xkh = wpool.tile([128, 512], F16, tag="wrk", name=f"xkh{c}")
                    nc.scalar.activation(xkh, kacc, ACT_COPY)
                    xks = wpool.tile([128, 512], F16, tag="wrk", name=f"xks{c}")
                    nc.vector.tensor_copy(xks[0:64, :], xkh[64:128, :])
                    nc.vector.tensor_copy(xks[64:128, :], xkh[0:64, :])
                    nc.vector.tensor_mul(xkh, xkh, cos_sb[:, c, 0, :])
                    nc.vector.tensor_mul(xks, xks, sin_sb[:, c, 0, :])
                    nc.vector.tensor_add(kt_sb[:, 512 * c : 512 * (c + 1)], xkh, xks)

                def fin_v():
                    vt = wpool.tile([128, 512], F16, tag="wrk", name=f"vt{c}")
                    nc.scalar.activation(vt, vacc, ACT_COPY)
                    for s in range(4):
                        ptr = ps.tile(
                            [128, 128], F16, tag="pa", bufs=4, name=f"vtr{c}_{s}"
                        )
                        nc.tensor.transpose(
                            ptr, vt[:, 128 * s : 128 * (s + 1)], iden_sb
                        )
                        nc.vector.tensor_copy(v_sb[:, 4 * c + s, :], ptr)

                for g in range(4):
                    pc = pieces[4 * c + g]
                    for oo in range(8):
                        o = 8 * g + oo
                        rtt = pc[:, oo, :]
                        st, sp = (o == 0), (o == 31)
                        for h in range(HQ):
                            nc.tensor.matmul(
                                accs[h],
                                wq_sb[:, o, 128 * h : 128 * (h + 1)],
                                rtt,
                                start=st,
                                stop=sp,
                            )
                            # finalize each pair-acc as soon as its last
                            # matmul is emitted so the ACT cast chain starts
                            # before the k/v matmuls finish
                            if sp and h == 1:
                                rope_q(0)
                            if sp and h == 3:
                                rope_q(1)
                        nc.tensor.matmul(kacc, wk_sb[:, o, :], rtt, start=st, stop=sp)
                        if sp:
                            rope_k()
                        nc.tensor.matmul(vacc, wv_sb[:, o, :], rtt, start=st, stop=sp)
                        if sp:
                            fin_v()
                        if micro is not None:
                            for fn in next(micro, []):
                                fn()
                    rt_dma(4 * c + g + 3)
                if micro is not None:
                    for step in micro:
                        for fn in step:
                            fn()

            # ---- attention chunk (with optional wo filler) ----
            def attn_chunk(c, filler):
                cs = slice(512 * c, 512 * (c + 1))
                npair = 2 * (c + 1)
                delayed = []  # deferred den/normalize thunks
                if filler is not None:
                    # bridge the rope-cast stall at the proj->attn boundary
                    # with independent PE work (keeps the HAM clock warm)
                    filler.emit(3)

                def run_delayed():
                    if delayed:
                        delayed.pop(0)()

                def mk_den(h, pacc, ctx_ps):
                    def den_thunk():
                        den_ps = ps.tile(
                            [1, 512], F32, tag="pa", bufs=4, name=f"den{c}_{h}"
                        )
                        nc.tensor.matmul(
                            den_ps, onesk_sb, pacc[:, 0, :], start=True, stop=False
                        )
                        nc.tensor.matmul(
                            den_ps, onesk_sb, pacc[:, 1, :], start=False, stop=True
                        )
                        # 1/den = exp(-ln(den)): Ln and Exp share one ACT
                        # table set, and the rank-1 matmul broadcasts ln(den)
                        # across partitions before the (vectorized) Exp.
                        lnden = spool.tile(
                            [1, 512], F32, tag="den", name=f"lnden{c}_{h}"
                        )
                        nc.scalar.activation(lnden, den_ps, ACT_LN)

                        def fin_thunk():
                            bc_ps = ps.tile(
                                [128, 512], F32, tag="pa", bufs=4, name=f"bc{c}_{h}"
                            )
                            nc.tensor.matmul(
                                bc_ps, onesf_sb, lnden, start=True, stop=True
                            )
                            bc_sb = spool.tile(
                                [128, 512], F32, tag="bcb", name=f"bcsb{c}_{h}"
                            )
                            nc.scalar.activation(bc_sb, bc_ps, ACT_EXP, scale=-1.0)
                            nc.vector.tensor_mul(cx_sb[:, h, cs], ctx_ps, bc_sb)

                        delayed.append(fin_thunk)

                    delayed.append(den_thunk)

                for h in range(HQ):
                    ctx_ps = ps.tile(
                        [128, 512], F32, tag="pa", bufs=4, name=f"ctx{c}_{h}"
                    )
                    pacc = papool.tile(
                        [128, 2, 512], F16, tag="pacc", name=f"pacc{c}_{h}"
                    )
                    prev = None
                    for pi in range(npair):
                        s2 = ps.tile(
                            [128, 2, 512], F32, tag="pb", bufs=2, name=f"s{c}_{h}_{pi}"
                        )
                        for hf in range(2):
                            j = 2 * pi + hf
                            nc.tensor.matmul(
                                s2[:, hf, :],
                                kt_sb[:, 128 * j : 128 * (j + 1)],
                                qt_sb[:, h, cs],
                                start=True,
                                stop=True,
                            )
                        p2 = p2pool.tile([128, 2, 512], F16, tag="p2", name=f"p{c}_{h}_{pi}")
                        nc.scalar.activation(p2, s2, ACT_EXP, scale=SCALE)
                        rp = pi - 2 * c
                        if rp >= 0:  # diagonal pair: causal mask
                            nc.vector.tensor_mul(p2, p2, msk_sb[:, rp, :, :])
                        if pi == 0:
                            nc.vector.tensor_copy(pacc, p2)
                        else:
                            nc.vector.tensor_add(pacc, pacc, p2)
                        if prev is not None:
                            pp2, ppi = prev
                            for hf in range(2):
                                j = 2 * ppi + hf
                                nc.tensor.matmul(
                                    ctx_ps,
                                    v_sb[:, j, :],
                                    pp2[:, hf, :],
                                    start=(j == 0),
                                    stop=False,
                                )
                        prev = (p2, pi)
                        if filler is not None:
                            filler.emit(1)
                        if pi >= 1:
                            # defer den/normalize chains one extra pair-slot so
                            # their matmuls never wait on the ACT/DVE chain
                            run_delayed()
                    pp2, ppi = prev
                    for hf in range(2):
                        j = 2 * ppi + hf
                        nc.tensor.matmul(
                            ctx_ps,
                            v_sb[:, j, :],
                            pp2[:, hf, :],
                            start=(j == 0),
                            stop=(hf == 1),
                        )
                    mk_den(h, pacc, ctx_ps)
                    if filler is not None:
                        filler.emit(1)
                    run_delayed()
                # drain deferred chains, with filler between to keep PE fed
                while delayed:
                    if filler is not None:
                        filler.emit(1)
                    run_delayed()
                if filler is not None:
                    filler.flush()

            # ---- attention chunk 0 as micro-steps inside P1's o-loop ----
            # single-tile scores (tag "pa"), one exp per k-tile; 7 steps per
            # head x 4 heads = 28 steps fit the 32 o-slots exactly.
            def attn0_micro():
                state = {}

                def mk_s(h, j):
                    def f():
                        if j == 0:
                            state["ctx"] = ps.tile(
                                [128, 512], F32, tag="pa", bufs=4, name=f"ctx0_{h}"
                            )
                            state["pacc"] = papool.tile(
                                [128, 512], F16, tag="pacc", name=f"pacc0_{h}"
                            )
                        s1 = ps.tile(
                            [128, 512], F32, tag="pa", bufs=4, name=f"s0_{h}_{j}"
                        )
                        nc.tensor.matmul(
                            s1,
                            kt_sb[:, 128 * j : 128 * (j + 1)],
                            qt_sb[:, h, 0:512],
                            start=True,
                            stop=True,
                        )
                        p1 = p2pool.tile(
                            [128, 512], F16, tag="p2", name=f"p0_{h}_{j}"
                        )
                        nc.scalar.activation(p1, s1, ACT_EXP, scale=SCALE)
                        nc.vector.tensor_mul(p1, p1, msk_sb[:, j // 2, j % 2, :])
                        if j == 0:
                            nc.vector.tensor_copy(state["pacc"], p1)
                        else:
                            nc.vector.tensor_add(state["pacc"], state["pacc"], p1)
                        state[("p", j)] = p1

                    return f

                def mk_ctx(h, j):
                    def f():
                        nc.tensor.matmul(
                            state["ctx"],
                            v_sb[:, j, :],
                            state[("p", j)],
                            start=(j == 0),
                            stop=(j == 3),
                        )

                    return f

                def mk_den(h):
                    def f():
                        den_ps = ps.tile(
                            [1, 512], F32, tag="pa", bufs=4, name=f"den0_{h}"
                        )
                        nc.tensor.matmul(
                            den_ps, onesk_sb, state["pacc"], start=True, stop=True
                        )
                        lnden = spool.tile([1, 512], F32, tag="den", name=f"lnd0_{h}")
                        nc.scalar.activation(lnden, den_ps, ACT_LN)
                        state["lnden"] = lnden

                    return f

                def mk_fin(h):
                    def f():
                        ctx_ps = state["ctx"]
                        bc_ps = ps.tile(
                            [128, 512], F32, tag="pa", bufs=4, name=f"bc0_{h}"
                        )
                        nc.tensor.matmul(
                            bc_ps, onesf_sb, state["lnden"], start=True, stop=True
                        )
                        bc_sb = spool.tile(
                            [128, 512], F32, tag="bcb", name=f"bcsb0_{h}"
                        )
                        nc.scalar.activation(bc_sb, bc_ps, ACT_EXP, scale=-1.0)
                        nc.vector.tensor_mul(cx_sb[:, h, 0:512], ctx_ps, bc_sb)

                    return f

                for h in range(HQ):
                    yield [mk_s(h, 0)]
                    for j in range(1, 4):
                        yield [mk_ctx(h, j - 1), mk_s(h, j)]
                    yield [mk_ctx(h, 3)]
                    yield [mk_den(h)]
                    yield [mk_fin(h)]

            # ---- emission: software-pipelined phases ----
            proj_chunk(0)
            # wo weights: overlap their DMA with chunk-1 compute
            nc.scalar.dma_start(wo_sb[:, 0:2, :], wo3[:, 0:2, :])
            proj_chunk(1, micro=attn0_micro())
            nc.scalar.dma_start(wo_sb[:, 2:4, :], wo3[:, 2:4, :])
            proj_chunk(2)
            attn_chunk(1, WoQueue(0))
            proj_chunk(3)
            attn_chunk(2, WoQueue(1))
            attn_chunk(3, WoQueue(2))
            w3 = WoQueue(3)
            w3.flush()

    # TRN2 allows at most 1 sem wait per instruction; split the extras into
    # EventSemaphore chains (same pass bacc.compile runs).
    bass_rust.generate_event_semaphores(nc)
    return nc


_NC = None


def _get_nc():
    global _NC
    if _NC is None:
        _NC = _build_nc()
    return _NC


def _host_inputs(resid, Wq, Wk, Wv, Wo):
    f16 = np.float16
    r2 = np.asarray(resid, dtype=np.float32).reshape(T, D_MODEL)
    rt = np.ascontiguousarray(r2.T).astype(f16)  # [D, T]
    cosT, sinM = _rope_tables()
    cos2 = np.empty((DH, NCH, 2, 512), np.float32)
    sin2 = np.empty((DH, NCH, 2, 512), np.float32)
    for c in range(NCH):
        for hf in range(2):
            cos2[:, c, hf, :] = cosT[:, 512 * c : 512 * (c + 1)]
            sin2[:, c, hf, :] = sinM[:, 512 * c : 512 * (c + 1)]
    cos2 = cos2.astype(f16)
    sin2 = sin2.astype(f16)
    # pair masks: msk[part, rp, hf, u] = part <= u - 128*(2*rp+hf)
    u = np.arange(512)[None, :]
    p = np.arange(128)[:, None]
    msk = np.empty((128, 2, 2, 512), np.float32)
    for rp in range(2):
        for hf in range(2):
            msk[:, rp, hf, :] = (p <= u - 128 * (2 * rp + hf)).astype(np.float32)
    msk = msk.astype(f16)
    iden = np.eye(128, dtype=f16)
    onesk = np.ones((128, 1), f16)
    onesf = np.ones((1, 128), np.float32)
    Wq = np.asarray(Wq, np.float32)
    Wk = np.asarray(Wk, np.float32)
    Wv = np.asarray(Wv, np.float32)
    Wo = np.asarray(Wo, np.float32)
    in_maps = []
    for i in range(NCORES):
        in_maps.append(
            {
                "rt": rt,
                "wq": np.ascontiguousarray(Wq[:, 512 * i : 512 * (i + 1)]).astype(f16),
                "wk": np.ascontiguousarray(Wk[:, 128 * i : 128 * (i + 1)]).astype(f16),
                "wv": np.ascontiguousarray(Wv[:, 128 * i : 128 * (i + 1)]).astype(f16),
                "wo": np.ascontiguousarray(Wo[512 * i : 512 * (i + 1), :]).astype(f16),
                "cos2": cos2,
                "sin2": sin2,
                "msk": msk,
                "iden": iden,
                "onesk": onesk,
                "onesf": onesf,
            }
        )
    return in_maps


def run(resid, Wq, Wk, Wv, Wo, **spmd_kwargs):
    in_maps = _host_inputs(resid, Wq, Wk, Wv, Wo)
    nc = _get_nc()
    res = run_bass_kernel_spmd(nc, in_maps, core_ids=list(range(NCORES)), **spmd_kwargs)
    out = np.zeros((T, D_MODEL), np.float32)
    for rmap in res.results:
        out += rmap["outp"].astype(np.float32)
    return out.reshape(1, T, D_MODEL), res


def kernel(resid, Wq, Wk, Wv, Wo):
    out, _ = run(resid, Wq, Wk, Wv, Wo)
    return out


# revision 22
# speedup vs baseline: 1.1010x; 1.0571x over previous
"""Llama3 GQA causal attention (B=1, T=2048, D=4096, 32 Q heads / 8 KV heads,
dh=128) on 8 Trainium2 NeuronCores.

Sharding: tensor-parallel over heads. Core i owns KV head i and Q heads
4i..4i+3: Wq/Wk/Wv split column-wise, Wo split row-wise. Each core computes a
partial [T, D] output (rows of Wo for its heads); the host sums the 8 partials.

v2 layout/scheduling notes (vs the original baseline):
 - all matmul operands in fp16 (same PE rate as bf16, ~8x less quantization
   error); PSUM accumulation stays fp32.
 - DMAs are batched: rt streams in [128, 8, 512] pieces (1MB), weights in a
   handful of large transfers. The SP engine issues ~90 DMAs instead of ~360
   (607ns of issue cost each was starving the projection phases).
 - scores are computed in [128, 2, 512] pair tiles so one ACT exp covers 1024
   columns (the 293ns/instr ACT overhead was pacing the attention phases).
 - softmax denominator: probs pairs are accumulated on the DVE (fp16) and
   reduced with 2 matmuls per head instead of one matmul per k-tile (saves
   ~28us of PE time).
 - 1/den via reciprocal_approx_fast (single DVE op) instead of the iterative
   reciprocal (3.3us on one partition).
 - emission interleaves Wo matmul groups into the attention phases as PE
   filler so the PE never waits on the exp chain; phases are software-
   pipelined: P0 P1 A0 P2 A1|W0 P3 A2|W1 A3|W2 W3.
 - PSUM plan: tag "pa" 4x one-bank slots + tag "pb" 2x two-bank slots = 8.
"""

import math
import sys

import numpy as np

sys.path.insert(0, "/opt/trn_rl_repo")

import bass_rust

import concourse.bass as bass
import concourse.mybir as mybir
import concourse.tile as tile
from concourse.bass_utils import run_bass_kernel_spmd

F16 = mybir.dt.float16
F32 = mybir.dt.float32
ACT_COPY = mybir.ActivationFunctionType.Copy
ACT_EXP = mybir.ActivationFunctionType.Exp
ACT_LN = mybir.ActivationFunctionType.Ln

D_MODEL = 4096
N_HEADS = 32
N_KV = 8
DH = 128
T = 2048
NCORES = 8
HQ = N_HEADS // NCORES  # 4 q heads per core
NT = T // 128  # 16 k tiles
NCH = T // 512  # 4 q chunks
SCALE = 1.0 / math.sqrt(DH)
ROPE = dict(
    rope_theta=500000.0,
    factor=32.0,
    hi_freq_factor=4.0,
    lo_freq_factor=1.0,
    original_context_length=8192,
)


def _rope_tables():
    """cos/sin tables in transposed layout [dh, T]; sin has the rotate-half
    sign folded in (rows 0:64 negated)."""
    idx = np.arange(0, DH, 2, dtype=np.float64) / DH
    freq = (1.0 / (2.0 * math.pi)) * ROPE["rope_theta"] ** (-idx)
    factor, lo, hi = ROPE["factor"], ROPE["lo_freq_factor"], ROPE["hi_freq_factor"]
    L0 = ROPE["original_context_length"]
    freq_low, freq_high = lo / L0, hi / L0
    freq_scaled = np.where(freq < freq_low, freq / factor, freq)
    smooth = np.clip((L0 * freq - lo) / (hi - lo), 0.0, 1.0)
    freq_smooth = (1.0 - smooth) * (freq / factor) + smooth * freq
    is_mid = (freq >= freq_low) & (freq <= freq_high)
    freq = np.where(is_mid, freq_smooth, freq_scaled)
    pos = np.arange(T, dtype=np.float64)
    phase = 2.0 * math.pi * pos[:, None] * freq[None, :]  # [T, 64]
    emb = np.concatenate([phase, phase], axis=-1)  # [T, 128]
    cos = np.cos(emb).astype(np.float32)
    sin = np.sin(emb).astype(np.float32)
    cosT = np.ascontiguousarray(cos.T)  # [128, T]
    sinT = np.ascontiguousarray(sin.T)
    sinM = sinT.copy()
    sinM[:64] = -sinT[:64]
    return cosT, sinM


def _build_nc():
    nc = bass.Bass()
    # all big inputs come pre-arranged partition-major on the host so every
    # DMA moves long (>=2KB) contiguous runs per partition
    rt = nc.dram_tensor("rt", [128, NCH, 32, 512], F16, kind="ExternalInput")
    wq = nc.dram_tensor("wq", [128, 32, HQ * DH], F16, kind="ExternalInput")
    wk = nc.dram_tensor("wk", [128, 32, DH], F16, kind="ExternalInput")
    wv = nc.dram_tensor("wv", [128, 32, DH], F16, kind="ExternalInput")
    wo = nc.dram_tensor("wo", [128, HQ, D_MODEL], F16, kind="ExternalInput")
    # duplicated-pair rope tables: [:, c, hf, :] = table[:, 512c:512c+512]
    cos2 = nc.dram_tensor("cos2", [DH, NCH, 2, 512], F16, kind="ExternalInput")
    sin2 = nc.dram_tensor("sin2", [DH, NCH, 2, 512], F16, kind="ExternalInput")
    # pair masks msk[:, rp, hf, :] for diagonal pair rp, half hf
    msk = nc.dram_tensor("msk", [128, 2, 2, 512], F16, kind="ExternalInput")
    iden = nc.dram_tensor("iden", [128, 128], F16, kind="ExternalInput")
    onesk = nc.dram_tensor("onesk", [128, 1], F16, kind="ExternalInput")
    onesf = nc.dram_tensor("onesf", [1, 128], F32, kind="ExternalInput")
    outp = nc.dram_tensor("outp", [T, D_MODEL], F16, kind="ExternalOutput")

    rt3, wq3, wk3, wv3, wo3 = rt, wq, wk, wv, wo

    with tile.TileContext(nc) as tc:
        with (
            tc.tile_pool(name="consts", bufs=1) as cpool,
            tc.tile_pool(name="acts", bufs=1) as apool,
            tc.tile_pool(name="rtp", bufs=3) as rpool,
            tc.tile_pool(name="wrk", bufs=4) as wpool,
            tc.tile_pool(name="p2p", bufs=3) as p2pool,
            tc.tile_pool(name="pap", bufs=2) as papool,
            tc.tile_pool(name="sml", bufs=2) as spool,
            tc.tile_pool(name="stgp", bufs=2) as opool,
            tc.tile_pool(name="ps", bufs=4, space="PSUM") as ps,
        ):
            # ---- resident constants / weights (few big DMAs) ----
            iden_sb = cpool.tile([128, 128], F16)
            nc.sync.dma_start(iden_sb, iden[:, :])
            onesk_sb = cpool.tile([128, 1], F16)
            nc.sync.dma_start(onesk_sb, onesk[:, :])
            onesf_sb = cpool.tile([1, 128], F32)
            nc.sync.dma_start(onesf_sb, onesf[:, :])
            msk_sb = cpool.tile([128, 2, 2, 512], F16)
            nc.sync.dma_start(msk_sb, msk[:, :, :, :])

            wq_sb = cpool.tile([128, 32, HQ * DH], F16)
            wk_sb = cpool.tile([128, 32, DH], F16)
            wv_sb = cpool.tile([128, 32, DH], F16)
            wo_sb = cpool.tile([128, HQ, D_MODEL], F16)
            cos_sb = cpool.tile([DH, NCH, 2, 512], F16)
            sin_sb = cpool.tile([DH, NCH, 2, 512], F16)

            # activations that persist across phases
            qt_sb = apool.tile([128, HQ, T], F16)  # Q^T per head, rope'd
            kt_sb = apool.tile([128, T], F16)  # K^T, rope'd
            v_sb = apool.tile([128, NT, DH], F16)  # V tiles [tk, j, dh]
            cx_sb = apool.tile([128, HQ, T], F16)  # normalized ctx^T

            # ---- rt piece DMA bookkeeping ----
            pieces = {}

            def rt_dma(p):
                if p >= 16:
                    return
                c, g = p // 4, p % 4
                t = rpool.tile([128, 8, 512], F16, tag="rtp", name=f"rt{c}_{g}")
                nc.sync.dma_start(t, rt3[:, c, 8 * g : 8 * (g + 1), :])
                pieces[p] = t

            # preamble: the sync ring streams exactly what the o-loop needs in
            # o-group-major order (wq/wk/wv slices + the rt piece per 8-o
            # group); rope tables and wo ride the scalar ring so they don't
            # delay the critical stream.
            for g in range(4):
                nc.sync.dma_start(
                    wq_sb[:, 8 * g : 8 * (g + 1), :], wq3[:, 8 * g : 8 * (g + 1), :]
                )
                nc.sync.dma_start(
                    wk_sb[:, 8 * g : 8 * (g + 1), :], wk3[:, 8 * g : 8 * (g + 1), :]
                )
                nc.sync.dma_start(
                    wv_sb[:, 8 * g : 8 * (g + 1), :], wv3[:, 8 * g : 8 * (g + 1), :]
                )
                if g < 3:
                    rt_dma(g)
            nc.sync.dma_start(cos_sb, cos2[:, :, :, :])
            nc.sync.dma_start(sin_sb, sin2[:, :, :, :])

            # PE warm-up: junk matmuls on tiny early-arriving constants fill
            # the initial DMA wait and flip the HAM clock gate to 8/8 before
            # the first real matmul lands.
            junk_ps = ps.tile([128, 2, 512], F32, tag="pb", bufs=2, name="junk")
            mskf = msk_sb.rearrange("p a b f -> p (a b f)")
            for w in range(16):
                nc.tensor.matmul(
                    junk_ps[:, w % 2, :],
                    iden_sb,
                    mskf[:, 512 * (w % 4) : 512 * (w % 4 + 1)],
                    start=True,
                    stop=True,
                )

            # ---- wo filler queue ----
            class WoQueue:
                def __init__(self, c):
                    self.c = c
                    self.items = [(4 * c + s, n) for s in range(4) for n in range(8)]
                    self.i = 0
                    self.stg = None

                def done(self):
                    return self.i >= len(self.items)

                def emit(self, k):
                    for _ in range(k):
                        if self.done():
                            return
                        tq, n = self.items[self.i]
                        self.i += 1
                        if n % 4 == 0:
                            self.stg = opool.tile(
                                [128, 2048], F16, tag="stg", name=f"stg{tq}_{n}"
                            )
                        wps = ps.tile(
                            [128, 512], F32, tag="pa", bufs=4, name=f"wo{tq}_{n}"
                        )
                        for h in range(HQ):
                            nc.tensor.matmul(
                                wps,
                                cx_sb[:, h, 128 * tq : 128 * (tq + 1)],
                                wo_sb[:, h, 512 * n : 512 * (n + 1)],
                                start=(h == 0),
                                stop=(h == HQ - 1),
                            )
                        dst = self.stg[:, 512 * (n % 4) : 512 * (n % 4 + 1)]
                        if n % 2 == 0:
                            nc.scalar.activation(dst, wps, ACT_COPY)
                        else:
                            nc.vector.tensor_copy(dst, wps)
                        if n % 4 == 3:
                            nc.sync.dma_start(
                                outp[
                                    128 * tq : 128 * (tq + 1),
                                    2048 * (n // 4) : 2048 * (n // 4 + 1),
                                ],
                                self.stg,
                            )

                def flush(self):
                    self.emit(len(self.items) - self.i)

            # ---- projection chunk ----
            def proj_chunk(c, micro=None, defer_v=False, pre=None):
                bq01 = ps.tile([128, 2, 512], F32, tag="pb", bufs=2, name=f"bq01_{c}")
                bq23 = ps.tile([128, 2, 512], F32, tag="pb", bufs=2, name=f"bq23_{c}")
                kacc = ps.tile([128, 512], F32, tag="pa", bufs=4, name=f"kacc{c}")
                vacc = ps.tile([128, 512], F32, tag="pa", bufs=4, name=f"vacc{c}")
                accs = [bq01[:, 0, :], bq01[:, 1, :], bq23[:, 0, :], bq23[:, 1, :]]

                def rope_q(bi):
                    big = [bq01, bq23][bi]
                    xh = wpool.tile(
                        [128, 2, 512], F16, tag="wrk", name=f"xh{c}_{bi}"
                    )
                    nc.scalar.activation(xh, big, ACT_COPY)
                    xs = wpool.tile(
                        [128, 2, 512], F16, tag="wrk", name=f"xs{c}_{bi}"
                    )
                    nc.vector.tensor_copy(xs[0:64, :, :], xh[64:128, :, :])
                    nc.vector.tensor_copy(xs[64:128, :, :], xh[0:64, :, :])
                    nc.vector.tensor_mul(xh, xh, cos_sb[:, c, :, :])
                    nc.vector.tensor_mul(xs, xs, sin_sb[:, c, :, :])
                    nc.vector.tensor_add(
                        qt_sb[:, 2 * bi : 2 * bi + 2, 512 * c : 512 * (c + 1)], xh, xs
                    )

                def rope_k():
                    xkh = wpool.tile([128, 512], F16, tag="wrk", name=f"xkh{c}")
                    nc.scalar.activation(xkh, kacc, ACT_COPY)
                    xks = wpool.tile([128, 512], F16, tag="wrk", name=f"xks{c}")
                    nc.vector.tensor_copy(xks[0:64, :], xkh[64:128, :])
                    nc.vector.tensor_copy(xks[64:128, :], xkh[0:64, :])
                    nc.vector.tensor_mul(xkh, xkh, cos_sb[:, c, 0, :])
                    nc.vector.tensor_mul(xks, xks, sin_sb[:, c, 0, :])
                    nc.vector.tensor_add(kt_sb[:, 512 * c : 512 * (c + 1)], xkh, xks)

                def fin_v():
                    vt = wpool.tile([128, 512], F16, tag="wrk", name=f"vt{c}")
                    nc.scalar.activation(vt, vacc, ACT_COPY)
                    for s in range(4):
                        ptr = ps.tile(
                            [128, 128], F16, tag="pa", bufs=4, name=f"vtr{c}_{s}"
                        )
                        nc.tensor.transpose(
                            ptr, vt[:, 128 * s : 128 * (s + 1)], iden_sb
                        )
                        nc.vector.tensor_copy(v_sb[:, 4 * c + s, :], ptr)

                for g in range(4):
                    pc = pieces[4 * c + g]
                    for oo in range(8):
                        o = 8 * g + oo
                        rtt = pc[:, oo, :]
                        st, sp = (o == 0), (o == 31)
                        for h in range(HQ):
                            nc.tensor.matmul(
                                accs[h],
                                wq_sb[:, o, 128 * h : 128 * (h + 1)],
                                rtt,
                                start=st,
                                stop=sp,
                            )
                            # finalize each pair-acc as soon as its last
                            # matmul is emitted so the ACT cast chain starts
                            # before the k/v matmuls finish
                            if sp and h == 1:
                                rope_q(0)
                            if sp and h == 3:
                                rope_q(1)
                        nc.tensor.matmul(kacc, wk_sb[:, o, :], rtt, start=st, stop=sp)
                        if sp:
                            rope_k()
                        nc.tensor.matmul(vacc, wv_sb[:, o, :], rtt, start=st, stop=sp)
                        if sp and not defer_v:
                            fin_v()
                        if micro is not None:
                            for fn in next(micro, []):
                                fn()
                        if o == 0 and pre is not None:
                            pre()
                    rt_dma(4 * c + g + 3)
                if micro is not None:
                    for step in micro:
                        for fn in step:
                            fn()
                return fin_v if defer_v else None

            # ---- attention chunk (with optional wo filler) ----
            def attn_chunk(c, filler, pre=None):
                cs = slice(512 * c, 512 * (c + 1))
                npair = 2 * (c + 1)
                delayed = []  # deferred den/normalize thunks
                if filler is not None:
                    # bridge the rope-cast stall at the proj->attn boundary
                    # with independent PE work (keeps the HAM clock warm)
                    filler.emit(3)
                if pre is not None:
                    pre()

                def run_delayed():
                    if delayed:
                        delayed.pop(0)()

                def mk_den(h, pacc, ctx_ps):
                    def den_thunk():
                        den_ps = ps.tile(
                            [1, 512], F32, tag="pa", bufs=4, name=f"den{c}_{h}"
                        )
                        nc.tensor.matmul(
                            den_ps, onesk_sb, pacc[:, 0, :], start=True, stop=False
                        )
                        nc.tensor.matmul(
                            den_ps, onesk_sb, pacc[:, 1, :], start=False, stop=True
                        )
                        # 1/den = exp(-ln(den)): Ln and Exp share one ACT
                        # table set, and the rank-1 matmul broadcasts ln(den)
                        # across partitions before the (vectorized) Exp.
                        lnden = spool.tile(
                            [1, 512], F32, tag="den", name=f"lnden{c}_{h}"
                        )
                        nc.scalar.activation(lnden, den_ps, ACT_LN)

                        def fin_thunk():
                            bc_ps = ps.tile(
                                [128, 512], F32, tag="pa", bufs=4, name=f"bc{c}_{h}"
                            )
                            nc.tensor.matmul(
                                bc_ps, onesf_sb, lnden, start=True, stop=True
                            )
                            bc_sb = spool.tile(
                                [128, 512], F32, tag="bcb", name=f"bcsb{c}_{h}"
                            )
                            nc.scalar.activation(bc_sb, bc_ps, ACT_EXP, scale=-1.0)
                            nc.vector.tensor_mul(cx_sb[:, h, cs], ctx_ps, bc_sb)

                        delayed.append(fin_thunk)

                    delayed.append(den_thunk)

                for h in range(HQ):
                    ctx_ps = ps.tile(
                        [128, 512], F32, tag="pa", bufs=4, name=f"ctx{c}_{h}"
                    )
                    pacc = papool.tile(
                        [128, 2, 512], F16, tag="pacc", name=f"pacc{c}_{h}"
                    )
                    prev = None
                    for pi in range(npair):
                        s2 = ps.tile(
                            [128, 2, 512], F32, tag="pb", bufs=2, name=f"s{c}_{h}_{pi}"
                        )
                        for hf in range(2):
                            j = 2 * pi + hf
                            nc.tensor.matmul(
                                s2[:, hf, :],
                                kt_sb[:, 128 * j : 128 * (j + 1)],
                                qt_sb[:, h, cs],
                                start=True,
                                stop=True,
                            )
                        p2 = p2pool.tile([128, 2, 512], F16, tag="p2", name=f"p{c}_{h}_{pi}")
                        nc.scalar.activation(p2, s2, ACT_EXP, scale=SCALE)
                        rp = pi - 2 * c
                        if rp >= 0:  # diagonal pair: causal mask
                            nc.vector.tensor_mul(p2, p2, msk_sb[:, rp, :, :])
                        if pi == 0:
                            nc.vector.tensor_copy(pacc, p2)
                        else:
                            nc.vector.tensor_add(pacc, pacc, p2)
                        if prev is not None:
                            pp2, ppi = prev
                            for hf in range(2):
                                j = 2 * ppi + hf
                                nc.tensor.matmul(
                                    ctx_ps,
                                    v_sb[:, j, :],
                                    pp2[:, hf, :],
                                    start=(j == 0),
                                    stop=False,
                                )
                        prev = (p2, pi)
                        if filler is not None:
                            filler.emit(1)
                        if pi >= 1:
                            # defer den/normalize chains one extra pair-slot so
                            # their matmuls never wait on the ACT/DVE chain
                            run_delayed()
                    pp2, ppi = prev
                    for hf in range(2):
                        j = 2 * ppi + hf
                        nc.tensor.matmul(
                            ctx_ps,
                            v_sb[:, j, :],
                            pp2[:, hf, :],
                            start=(j == 0),
                            stop=(hf == 1),
                        )
                    mk_den(h, pacc, ctx_ps)
                    if filler is not None:
                        filler.emit(1)
                    run_delayed()
                # drain deferred chains, with filler between to keep PE fed
                while delayed:
                    if filler is not None:
                        filler.emit(1)
                    run_delayed()
                if filler is not None:
                    filler.flush()

            # ---- attention chunk 0 as micro-steps inside P1's o-loop ----
            # single-tile scores (tag "pa"), one exp per k-tile; 7 steps per
            # head x 4 heads = 28 steps fit the 32 o-slots exactly.
            def attn0_micro():
                state = {}

                def mk_s(h, j):
                    def f():
                        if j == 0:
                            state["ctx"] = ps.tile(
                                [128, 512], F32, tag="pa", bufs=4, name=f"ctx0_{h}"
                            )
                            state["pacc"] = papool.tile(
                                [128, 512], F16, tag="pacc", name=f"pacc0_{h}"
                            )
                        s1 = ps.tile(
                            [128, 512], F32, tag="pa", bufs=4, name=f"s0_{h}_{j}"
                        )
                        nc.tensor.matmul(
                            s1,
                            kt_sb[:, 128 * j : 128 * (j + 1)],
                            qt_sb[:, h, 0:512],
                            start=True,
                            stop=True,
                        )
                        p1 = p2pool.tile(
                            [128, 512], F16, tag="p2", name=f"p0_{h}_{j}"
                        )
                        nc.scalar.activation(p1, s1, ACT_EXP, scale=SCALE)
                        nc.vector.tensor_mul(p1, p1, msk_sb[:, j // 2, j % 2, :])
                        if j == 0:
                            nc.vector.tensor_copy(state["pacc"], p1)
                        else:
                            nc.vector.tensor_add(state["pacc"], state["pacc"], p1)
                        state[("p", j)] = p1

                    return f

                def mk_ctx(h, j):
                    def f():
                        nc.tensor.matmul(
                            state["ctx"],
                            v_sb[:, j, :],
                            state[("p", j)],
                            start=(j == 0),
                            stop=(j == 3),
                        )

                    return f

                def mk_den(h):
                    def f():
                        den_ps = ps.tile(
                            [1, 512], F32, tag="pa", bufs=4, name=f"den0_{h}"
                        )
                        nc.tensor.matmul(
                            den_ps, onesk_sb, state["pacc"], start=True, stop=True
                        )
                        lnden = spool.tile([1, 512], F32, tag="den", name=f"lnd0_{h}")
                        nc.scalar.activation(lnden, den_ps, ACT_LN)
                        state["lnden"] = lnden

                    return f

                def mk_fin(h):
                    def f():
                        ctx_ps = state["ctx"]
                        bc_ps = ps.tile(
                            [128, 512], F32, tag="pa", bufs=4, name=f"bc0_{h}"
                        )
                        nc.tensor.matmul(
                            bc_ps, onesf_sb, state["lnden"], start=True, stop=True
                        )
                        bc_sb = spool.tile(
                            [128, 512], F32, tag="bcb", name=f"bcsb0_{h}"
                        )
                        nc.scalar.activation(bc_sb, bc_ps, ACT_EXP, scale=-1.0)
                        nc.vector.tensor_mul(cx_sb[:, h, 0:512], ctx_ps, bc_sb)

                    return f

                for h in range(HQ):
                    yield [mk_s(h, 0)]
                    for j in range(1, 4):
                        yield [mk_ctx(h, j - 1), mk_s(h, j)]
                    yield [mk_ctx(h, 3)]
                    yield [mk_den(h)]
                    yield [mk_fin(h)]

            # ---- emission: software-pipelined phases ----
            proj_chunk(0)
            # wo weights: sync-ring position after chunk-0's rt pieces so the
            # scheduler cannot hoist them ahead of the critical startup stream
            nc.sync.dma_start(wo_sb[:, 0:2, :], wo3[:, 0:2, :])
            fv1 = proj_chunk(1, micro=attn0_micro(), defer_v=True)
            nc.sync.dma_start(wo_sb[:, 2:4, :], wo3[:, 2:4, :])
            fv2 = proj_chunk(2, defer_v=True, pre=fv1)
            attn_chunk(1, WoQueue(0), pre=fv2)
            fv3 = proj_chunk(3, defer_v=True)
            attn_chunk(2, WoQueue(1), pre=fv3)
            attn_chunk(3, WoQueue(2))
            w3 = WoQueue(3)
            w3.flush()

    # TRN2 allows at most 1 sem wait per instruction; split the extras into
    # EventSemaphore chains (same pass bacc.compile runs).
    bass_rust.generate_event_semaphores(nc)
    return nc


_NC = None


def _get_nc():
    global _NC
    if _NC is None:
        _NC = _build_nc()
    return _NC


def _host_inputs(resid, Wq, Wk, Wv, Wo):
    f16 = np.float16
    r2 = np.asarray(resid, dtype=np.float32).reshape(T, D_MODEL)
    # rt pre-arranged partition-major: [p, chunk, o, t'] so each DMA piece is
    # one contiguous 8KB run per partition
    rt = np.ascontiguousarray(
        r2.T.reshape(32, 128, NCH, 512).transpose(1, 2, 0, 3)
    ).astype(f16)  # [128, 4, 32, 512]
    cosT, sinM = _rope_tables()
    cos2 = np.empty((DH, NCH, 2, 512), np.float32)
    sin2 = np.empty((DH, NCH, 2, 512), np.float32)
    for c in range(NCH):
        for hf in range(2):
            cos2[:, c, hf, :] = cosT[:, 512 * c : 512 * (c + 1)]
            sin2[:, c, hf, :] = sinM[:, 512 * c : 512 * (c + 1)]
    cos2 = cos2.astype(f16)
    sin2 = sin2.astype(f16)
    # pair masks: msk[part, rp, hf, u] = part <= u - 128*(2*rp+hf)
    u = np.arange(512)[None, :]
    p = np.arange(128)[:, None]
    msk = np.empty((128, 2, 2, 512), np.float32)
    for rp in range(2):
        for hf in range(2):
            msk[:, rp, hf, :] = (p <= u - 128 * (2 * rp + hf)).astype(np.float32)
    msk = msk.astype(f16)
    iden = np.eye(128, dtype=f16)
    onesk = np.ones((128, 1), f16)
    onesf = np.ones((1, 128), np.float32)
    Wq = np.asarray(Wq, np.float32)
    Wk = np.asarray(Wk, np.float32)
    Wv = np.asarray(Wv, np.float32)
    Wo = np.asarray(Wo, np.float32)
    def pmajor(w, m):  # [D, m] -> [128, 32, m] partition-major
        return np.ascontiguousarray(w.reshape(32, 128, m).transpose(1, 0, 2)).astype(
            f16
        )

    in_maps = []
    for i in range(NCORES):
        wo_i = Wo[512 * i : 512 * (i + 1), :]
        in_maps.append(
            {
                "rt": rt,
                "wq": pmajor(Wq[:, 512 * i : 512 * (i + 1)], 512),
                "wk": pmajor(Wk[:, 128 * i : 128 * (i + 1)], 128),
                "wv": pmajor(Wv[:, 128 * i : 128 * (i + 1)], 128),
                "wo": np.ascontiguousarray(
                    wo_i.reshape(4, 128, D_MODEL).transpose(1, 0, 2)
                ).astype(f16),
                "cos2": cos2,
                "sin2": sin2,
                "msk": msk,
                "iden": iden,
                "onesk": onesk,
                "onesf": onesf,
            }
        )
    return in_maps


def run(resid, Wq, Wk, Wv, Wo, **spmd_kwargs):
    in_maps = _host_inputs(resid, Wq, Wk, Wv, Wo)
    nc = _get_nc()
    res = run_bass_kernel_spmd(nc, in_maps, core_ids=list(range(NCORES)), **spmd_kwargs)
    out = np.zeros((T, D_MODEL), np.float32)
    for rmap in res.results:
        out += rmap["outp"].astype(np.float32)
    return out.reshape(1, T, D_MODEL), res


def kernel(resid, Wq, Wk, Wv, Wo):
    out, _ = run(resid, Wq, Wk, Wv, Wo)
    return out
